# revision 6
# baseline (speedup 1.0000x reference)
"""Trainium2 Bass kernel for nn_CorrBlock: softmax(fmap1 @ fmap2.T / sqrt(D), axis=-1).

Sharding: fmap1 rows split across 8 cores (1024 rows each), fmap2 replicated.
Each core computes its [1024, 8192] slab of the output independently.

Device kernel (per core):
  - Inputs are pre-transposed on the host to [128, D/128, rows] so the
    contraction dim lands on SBUF partitions with no on-device transpose.
  - PE: matmuls accumulate the D=256 contraction in 2 chunks of 128 into PSUM.
  - ACT: Exp with fused 1/sqrt(D) scale reads PSUM, writes fp16 SBUF. No
    accum_out: the 187ns/chunk accumulator read would extend the ACT stream,
    and ACT is the critical engine (~60us of pure Exp at 1 elem/lane/cycle).
  - DVE: per-chunk row-sum reduces (fp16 in SBUF -> 4x mode), reciprocal,
    then per-row scalar multiply in fp16 (4x mode).
  - DMA out the normalized [128, 8192] block as fp16; host converts to f32.
    (fp16 halves output HBM traffic vs the f32 baseline, which was DMA-bound
    at 82%; softmax values round-trip fp16 at ~5e-4 rel err, well inside the
    2e-2 gate.)

Pipeline shaping (from v2 ntff trace):
  - f1 is loaded per 128-row block and f2 in 1024-col pieces so the first
    chunk's matmuls start as soon as ~0.6MB has landed instead of waiting
    for the full 1.5MB f1+f2-chunk prefix (lead-in was 17.3us of the 94us
    total; first matmul waited on serialized input DMA).
  - The last block's multiply+store is split into 1024-col pieces so the
    final DMA drain after the last Exp is shorter.
"""

import os
import sys

import numpy as np

if "/opt/trn_rl_repo" not in sys.path:
    sys.path.insert(0, "/opt/trn_rl_repo")

import concourse.bacc as bacc
import concourse.bass as bass
import concourse.mybir as mybir
import concourse.tile as tile
from concourse.bass_utils import run_bass_kernel_spmd

N, M, D = 8192, 8192, 256
N_CORES = 8
NB = N // N_CORES  # rows per core
DC = D // 128  # contraction chunks
QC = 2048  # columns handled per PSUM tile (4 banks)
PC = 1024  # columns per f2 input-load piece

# Matmul input dtype: "float16" halves input DMA bytes and doubles PE rate
# vs "float32r", at ~5e-4 softmax rel err (vs ~2e-4). Both are far inside
# tolerance; float16 wins on the DMA roofline.
MM_DT = os.environ.get("CORR_MM_DT", "float16")

# Populated by kernel() on every run (exec_time_ns only when tracing).
last_run_info: dict = {}


def build_nc(nb=NB, m=M, dc=DC, qc=QC, mm_dt=None, exp_bufs=4):
    """Build the per-core Bass program. Shapes in elements."""
    f32 = mybir.dt.float32
    f16 = mybir.dt.float16
    mm_dtype = getattr(mybir.dt, mm_dt or MM_DT)
    n_blocks = nb // 128
    n_q = m // qc  # PSUM-sized column chunks per row block
    n_j = qc // 512  # 512-wide matmul tiles per chunk
    n_p = m // PC  # f2 load pieces
    scale = 1.0 / (D**0.5)

    nc = bacc.Bacc("TRN2", target_bir_lowering=False, debug=False)

    f1t = nc.dram_tensor("f1t", [128, dc, nb], mm_dtype, kind="ExternalInput")
    f2t = nc.dram_tensor("f2t", [128, dc, m], mm_dtype, kind="ExternalInput")
    out = nc.dram_tensor("out", [nb, m], f16, kind="ExternalOutput")

    with tile.TileContext(nc) as tc:
        with (
            tc.tile_pool(name="weights", bufs=1) as wpool,
            tc.tile_pool(name="exps", bufs=exp_bufs) as epool,
            tc.tile_pool(name="stats", bufs=2) as spool,
            tc.tile_pool(name="psum", bufs=2, space="PSUM") as ppool,
        ):
            # Input loads, ordered so the first block's first chunk is ready
            # ASAP: f1 block 0, f2 pieces for chunk 0, remaining f1 blocks,
            # remaining f2 pieces.
            f1s = [
                wpool.tile([128, dc, 128], mm_dtype, tag=f"f1b_{b}", name=f"f1b_{b}")
                for b in range(n_blocks)
            ]
            f2s = [
                wpool.tile([128, dc, PC], mm_dtype, tag=f"f2p_{p}", name=f"f2p_{p}")
                for p in range(n_p)
            ]
            first_pieces = qc // PC  # pieces covering chunk 0
            nc.sync.dma_start(f1s[0][:], f1t[:, :, 0:128])
            for p in range(first_pieces):
                nc.sync.dma_start(f2s[p][:], f2t[:, :, p * PC : (p + 1) * PC])
            for b in range(1, n_blocks):
                nc.sync.dma_start(f1s[b][:], f1t[:, :, b * 128 : (b + 1) * 128])
            for p in range(first_pieces, n_p):
                nc.sync.dma_start(f2s[p][:], f2t[:, :, p * PC : (p + 1) * PC])

            jpp = PC // 512  # matmul j-tiles per f2 piece

            for b in range(n_blocks):
                exps = epool.tile([128, m], f16, tag="exps", name=f"exps_{b}")
                sums = spool.tile([128, n_q], f32, tag="sums", name=f"sums_{b}")
                rsum = spool.tile([128, 1], f32, tag="rsum", name=f"rsum_{b}")
                recip = spool.tile([128, 1], f32, tag="recip", name=f"recip_{b}")
                for q in range(n_q):
                    ps = ppool.tile([128, n_j, 512], f32, tag="ps", name=f"ps_{b}_{q}")
                    for d in range(dc):
                        lhsT = f1s[b][:, d, :]
                        for j in range(n_j):
                            jj = q * n_j + j  # global 512-tile index
                            piece = f2s[jj // jpp]
                            js = (jj % jpp) * 512
                            nc.tensor.matmul(
                                ps[:, j, :],
                                lhsT,
                                piece[:, d, js : js + 512],
                                start=(d == 0),
                                stop=(d == dc - 1),
                            )
                    sl = slice(q * qc, (q + 1) * qc)
                    nc.scalar.activation(
                        exps[:, sl],
                        ps.rearrange("p a b -> p (a b)"),
                        mybir.ActivationFunctionType.Exp,
                        scale=scale,
                    )
                    nc.vector.reduce_sum(
                        sums[:, q : q + 1], exps[:, sl], axis=mybir.AxisListType.X
                    )
                nc.vector.reduce_sum(rsum[:], sums[:], axis=mybir.AxisListType.X)
                nc.vector.reciprocal(recip[:], rsum[:])
                # Finer pieces on the last block shorten the final DMA drain.
                oc = qc if b < n_blocks - 1 else PC
                for o in range(m // oc):
                    sl = slice(o * oc, (o + 1) * oc)
                    nc.vector.tensor_scalar_mul(exps[:, sl], exps[:, sl], recip[:])
                    nc.sync.dma_start(out[b * 128 : (b + 1) * 128, sl], exps[:, sl])

    nc.compile()
    return nc


_nc_cache: dict = {}


def _get_nc():
    key = MM_DT
    if key not in _nc_cache:
        _nc_cache[key] = build_nc()
    return _nc_cache[key]


def kernel(fmap1: np.ndarray, fmap2: np.ndarray) -> np.ndarray:
    f1 = np.asarray(fmap1, dtype=np.float32)
    f2 = np.asarray(fmap2, dtype=np.float32)
    np_mm = mybir.dt.np(getattr(mybir.dt, MM_DT))
    # [rows, D] -> [128, D/128, rows]: f1t[dp, dcc, n] = f1[n, dcc*128 + dp]
    f1t = np.ascontiguousarray(
        f1.T.reshape(DC, 128, N).transpose(1, 0, 2).astype(np_mm)
    )
    f2t = np.ascontiguousarray(
        f2.T.reshape(DC, 128, M).transpose(1, 0, 2).astype(np_mm)
    )

    nc = _get_nc()
    in_maps = [
        {"f1t": np.ascontiguousarray(f1t[:, :, i * NB : (i + 1) * NB]), "f2t": f2t}
        for i in range(N_CORES)
    ]
    trace = bool(os.environ.get("BASS_TRACE"))
    res = run_bass_kernel_spmd(nc, in_maps, list(range(N_CORES)), trace=trace)
    last_run_info.clear()
    last_run_info.update(
        exec_time_ns=res.exec_time_ns,
        mean_exec_time_ns=res.mean_exec_time_ns,
        profile_json=res.profile_json,
        trace_path=(res.instructions_and_trace or (None, None))[1],
    )
    return np.concatenate(
        [res.results[i]["out"] for i in range(N_CORES)], axis=0
    ).astype(np.float32)


# revision 10
# speedup vs baseline: 1.2768x; 1.2768x over previous
"""Trainium2 Bass kernel for nn_CorrBlock: softmax(fmap1 @ fmap2.T / sqrt(D), axis=-1).

Sharding: fmap1 rows split across 8 cores (1024 rows each), fmap2 replicated.
Each core computes its [1024, 8192] slab of the output independently.

Device kernel (per core):
  - Inputs are pre-transposed on the host to [128, D/128, rows] so the
    contraction dim lands on SBUF partitions with no on-device transpose.
  - PE: matmuls accumulate the D=256 contraction in 2 chunks of 128 into PSUM.
  - ACT: Exp with fused 1/sqrt(D) scale reads PSUM, writes fp16 SBUF, and
    emits per-row partial sums via accum_out (f32). Computing the row sums
    as DVE reduces instead was tried and regressed hard: TENSOR_REDUCE gets
    no 2x/4x DVE mode (~2.3us per 2048-chunk, 74us total), overloading DVE,
    and its fp16 accumulation cost 10x on elementwise error. The 187ns/chunk
    accumulator read on ACT is the cheapest correct option.
  - DVE: reciprocal of the row sum, then per-row scalar multiply in fp16
    (4x mode, ~0.7us per 2048-chunk).
  - DMA out the normalized [128, 8192] block as fp16; host converts to f32.
    (fp16 halves output HBM traffic vs the f32 baseline, which was DMA-bound
    at 82%; softmax values round-trip fp16 at ~5e-4 rel err, well inside the
    2e-2 gate.)

Pipeline shaping (from v2 ntff trace):
  - f1 is loaded per 128-row block and f2 in 1024-col pieces so the first
    chunk's matmuls start as soon as ~0.6MB has landed instead of waiting
    for the full 1.5MB f1+f2-chunk prefix (lead-in was 17.3us of the 94us
    total; first matmul waited on serialized input DMA).
  - The last block's multiply+store is split into 1024-col pieces so the
    final DMA drain after the last Exp is shorter.
"""

import os
import sys

import numpy as np

if "/opt/trn_rl_repo" not in sys.path:
    sys.path.insert(0, "/opt/trn_rl_repo")

import concourse.bacc as bacc
import concourse.bass as bass
import concourse.mybir as mybir
import concourse.tile as tile
from concourse.bass_utils import run_bass_kernel_spmd

N, M, D = 8192, 8192, 256
N_CORES = 8
NB = N // N_CORES  # rows per core
DC = D // 128  # contraction chunks
QC = 2048  # columns handled per PSUM tile (4 banks)
PC = 1024  # columns per f2 input-load piece

# Matmul input dtype: "float16" halves input DMA bytes and doubles PE rate
# vs "float32r", at ~5e-4 softmax rel err (vs ~2e-4). Both are far inside
# tolerance; float16 wins on the DMA roofline.
MM_DT = os.environ.get("CORR_MM_DT", "float16")

# Populated by kernel() on every run (exec_time_ns only when tracing).
last_run_info: dict = {}


def build_nc(nb=NB, m=M, dc=DC, qc=QC, mm_dt=None, exp_bufs=5):
    """Build the per-core Bass program. Shapes in elements."""
    f32 = mybir.dt.float32
    f16 = mybir.dt.float16
    mm_dtype = getattr(mybir.dt, mm_dt or MM_DT)
    n_blocks = nb // 128
    n_q = m // qc  # PSUM-sized column chunks per row block
    n_j = qc // 512  # 512-wide matmul tiles per chunk
    n_p = m // PC  # f2 load pieces
    scale = 1.0 / (D**0.5)

    nc = bacc.Bacc("TRN2", target_bir_lowering=False, debug=False)

    f1t = nc.dram_tensor("f1t", [128, dc, nb], mm_dtype, kind="ExternalInput")
    f2t = nc.dram_tensor("f2t", [128, dc, m], mm_dtype, kind="ExternalInput")
    out = nc.dram_tensor("out", [nb, m], f16, kind="ExternalOutput")

    with tile.TileContext(nc) as tc:
        with (
            tc.tile_pool(name="weights", bufs=1) as wpool,
            tc.tile_pool(name="exps", bufs=exp_bufs) as epool,
            tc.tile_pool(name="stats", bufs=2) as spool,
            tc.tile_pool(name="psum", bufs=2, space="PSUM") as ppool,
        ):
            # Input loads, ordered so the first block's first chunk is ready
            # ASAP: f1 block 0, f2 pieces for chunk 0, remaining f1 blocks,
            # remaining f2 pieces.
            f1s = [
                wpool.tile([128, dc, 128], mm_dtype, tag=f"f1b_{b}", name=f"f1b_{b}")
                for b in range(n_blocks)
            ]
            f2s = [
                wpool.tile([128, dc, PC], mm_dtype, tag=f"f2p_{p}", name=f"f2p_{p}")
                for p in range(n_p)
            ]
            # Block 0 consumes all of f2 within its ~8us of ACT pacing, so the
            # f2 pieces must stream monotonically; the tiny per-block f1 loads
            # are interleaved after the first chunk's pieces so they never
            # gate a block start.
            nc.sync.dma_start(f1s[0][:], f1t[:, :, 0:128])
            loads = [0, 1, 2, 3]  # chunk 0+1 pieces first
            fb = 1

            for p in loads:
                nc.sync.dma_start(f2s[p][:], f2t[:, :, p * PC : (p + 1) * PC])
            for p in range(4, n_p):
                if fb < n_blocks:
                    nc.sync.dma_start(
                        f1s[fb][:], f1t[:, :, fb * 128 : (fb + 1) * 128]
                    )
                    fb += 1
                nc.sync.dma_start(f2s[p][:], f2t[:, :, p * PC : (p + 1) * PC])
            for b in range(fb, n_blocks):
                nc.sync.dma_start(f1s[b][:], f1t[:, :, b * 128 : (b + 1) * 128])

            jpp = PC // 512  # matmul j-tiles per f2 piece

            for b in range(n_blocks):
                exps = epool.tile([128, m], f16, tag="exps", name=f"exps_{b}")
                sums = spool.tile([128, n_q], f32, tag="sums", name=f"sums_{b}")
                rsum = spool.tile([128, 1], f32, tag="rsum", name=f"rsum_{b}")
                recip = spool.tile([128, 1], f32, tag="recip", name=f"recip_{b}")
                for q in range(n_q):
                    ps = ppool.tile([128, n_j, 512], f32, tag="ps", name=f"ps_{b}_{q}")
                    for d in range(dc):
                        lhsT = f1s[b][:, d, :]
                        for j in range(n_j):
                            jj = q * n_j + j  # global 512-tile index
                            piece = f2s[jj // jpp]
                            js = (jj % jpp) * 512
                            nc.tensor.matmul(
                                ps[:, j, :],
                                lhsT,
                                piece[:, d, js : js + 512],
                                start=(d == 0),
                                stop=(d == dc - 1),
                            )
                    sl = slice(q * qc, (q + 1) * qc)
                    nc.scalar.activation(
                        exps[:, sl],
                        ps.rearrange("p a b -> p (a b)"),
                        mybir.ActivationFunctionType.Exp,
                        scale=scale,
                        accum_out=sums[:, q : q + 1],
                    )
                nc.vector.reduce_sum(rsum[:], sums[:], axis=mybir.AxisListType.X)
                nc.vector.reciprocal(recip[:], rsum[:])
                # Finer pieces on the last block shorten the final DMA drain.
                oc = qc if b < n_blocks - 1 else PC
                for o in range(m // oc):
                    sl = slice(o * oc, (o + 1) * oc)
                    nc.vector.tensor_scalar_mul(exps[:, sl], exps[:, sl], recip[:])
                    nc.sync.dma_start(out[b * 128 : (b + 1) * 128, sl], exps[:, sl])

    nc.compile()
    return nc


_nc_cache: dict = {}


def _get_nc():
    key = MM_DT
    if key not in _nc_cache:
        _nc_cache[key] = build_nc()
    return _nc_cache[key]


def kernel(fmap1: np.ndarray, fmap2: np.ndarray) -> np.ndarray:
    f1 = np.asarray(fmap1, dtype=np.float32)
    f2 = np.asarray(fmap2, dtype=np.float32)
    np_mm = mybir.dt.np(getattr(mybir.dt, MM_DT))
    # [rows, D] -> [128, D/128, rows]: f1t[dp, dcc, n] = f1[n, dcc*128 + dp]
    f1t = np.ascontiguousarray(
        f1.T.reshape(DC, 128, N).transpose(1, 0, 2).astype(np_mm)
    )
    f2t = np.ascontiguousarray(
        f2.T.reshape(DC, 128, M).transpose(1, 0, 2).astype(np_mm)
    )

    nc = _get_nc()
    in_maps = [
        {"f1t": np.ascontiguousarray(f1t[:, :, i * NB : (i + 1) * NB]), "f2t": f2t}
        for i in range(N_CORES)
    ]
    trace = bool(os.environ.get("BASS_TRACE"))
    res = run_bass_kernel_spmd(nc, in_maps, list(range(N_CORES)), trace=trace)
    last_run_info.clear()
    last_run_info.update(
        exec_time_ns=res.exec_time_ns,
        mean_exec_time_ns=res.mean_exec_time_ns,
        profile_json=res.profile_json,
        trace_path=(res.instructions_and_trace or (None, None))[1],
    )
    return np.concatenate(
        [res.results[i]["out"] for i in range(N_CORES)], axis=0
    ).astype(np.float32)
